# revision 11
# baseline (speedup 1.0000x reference)
"""Trainium2 Bass kernel for the CDE+ODE problem (nn_CDEODE).

Reference: B=1024 trajectories; 59 fixed-step Tsit5 steps of a controlled
CDE (dy/dt = F(y) @ dX/dt, F: 3->128->128->128->9 tanh MLP, all-tanh) then
40 Tsit5 steps of a free ODE (dy/dt = G([y,t]), G: 4->128->128->128->3).
Output = channel 1 of the state at the 100 grid points.

Mapping: pure data-parallel over 8 cores (128 trajectories each).
Activations are feature-major [feat, batch]; each MLP layer is one fp32
PE matmul (lhsT = W.T) + one ACT tanh with fused per-partition bias.
All Tsit5 stage combinations are folded into the stage's first-layer
matmul: a "K buffer" SBUF tile pair holds [y; stage products] at engine-
legal partition bases (0/32/64/96) and precomputed fused weight matrices
(zero rows over the padding gaps) turn stage-input formation into plain
PSUM-accumulated matmuls.  The state update y_{t+1} is likewise two
small matmuls off the critical path; channel 1 is DMA'd out per step.
"""

import os
import sys

import numpy as np

for _p in ("/opt/trn_rl_repo",):
    if _p not in sys.path and os.path.isdir(_p):
        sys.path.append(_p)

import concourse.bacc as bacc
import concourse.mybir as mybir
import concourse.tile as tile
from concourse.bass_utils import run_bass_kernel_spmd

F32 = mybir.dt.float32
F16 = mybir.dt.float16
Tanh = mybir.ActivationFunctionType.Tanh

# precision mode: "fp32" everywhere, or "mixed" = fp16 weights/activations for
# the three wide layers (L2/L3/L4); stage/state matmuls stay fp32.
# fp32: HW-verified absmax 1.5e-5 vs reference; mixed: absmax 6.1e-3, ~3% faster.
PRECISION = "fp32"

NCORES = 8
B = 1024
BSH = B // NCORES  # 128 per core
L = 100
CU = 60   # control_until
TU = 100  # train_until
N1 = CU - 1        # phase-1 steps (59)
N2 = TU - CU       # phase-2 steps (40)
NSTEP = N1 + N2    # 99
W = 128            # MLP hidden width

# K-buffer geometry: block j (j=0 is y, j=1..6 stage products) lives in
# tile A or B at partition base 0/32/64/96.
#   phase 1: products are 9 rows;  A: y@0, p1@32, p2@64, p3@96 (105 rows)
#                                  B: p4@0, p5@32, p6@64      (73 rows)
#   phase 2: products are 3 rows;  A: y@0, k1@32, k2@64, k3@96 (99 rows)
#                                  B: k4@0, k5@32, k6@64      (67 rows)
KA1, KB1 = 105, 73
KA2, KB2 = 99, 67


def _blk(phase, j):
    """(tile_idx 0=A,1=B, row_base, nrows) for block j (0=y, 1..6=prods)."""
    if j == 0:
        return 0, 0, 3
    nr = 9 if phase == 1 else 3
    if j <= 3:
        return 0, 32 * j, nr
    return 1, 32 * (j - 4), nr


# Tsit5 tableau
_A = {
    (2, 1): 0.161,
    (3, 1): -0.008480655492356989, (3, 2): 0.335480655492357,
    (4, 1): 2.8971530571054935, (4, 2): -6.359448489975075, (4, 3): 4.3622954328695815,
    (5, 1): 5.325864828439257, (5, 2): -11.748883564062828, (5, 3): 7.4955393428898365,
    (5, 4): -0.09249506636175525,
    (6, 1): 5.86145544294642, (6, 2): -12.92096931784711, (6, 3): 8.159367898576159,
    (6, 4): -0.071584973281401, (6, 5): -0.028269050394068383,
}
_B = (0.09646076681806523, 0.01, 0.4798896504144996,
      1.379008574103742, -3.290069515436081, 2.324710524099774)
_C = (0.0, 0.161, 0.327, 0.9, 0.9800255409045097, 1.0)


# --------------------------------------------------------------------------
# host-side precompute
# --------------------------------------------------------------------------

def _fused_consts(func_params, ode_params, ts):
    """All weight-derived constant tensors (shared across cores), float32.

    Fused stage matrices map the padded K-buffer (A or B tile) to a layer-1
    pre-activation (or to the 3-row state update).  coef[j] is the scalar
    multiplying block j's contribution; block 0 (y) always has coef 1.
    """
    fW = [np.asarray(w, np.float64) for w, b in func_params]
    fb = [np.asarray(b, np.float64) for w, b in func_params]
    oW = [np.asarray(w, np.float64) for w, b in ode_params]
    ob = [np.asarray(b, np.float64) for w, b in ode_params]
    ts64 = np.asarray(ts, np.float64)
    dt = float(np.diff(ts64).mean())

    # phase-1 output permutation r = 3j+i <- f = 3i+j
    perm = np.empty(9, np.int64)
    for i in range(3):
        for j in range(3):
            perm[3 * j + i] = 3 * i + j
    W4p = fW[3][perm]
    b4p = fb[3][perm]
    S = np.concatenate([np.eye(3)] * 3, axis=0)   # [9,3]

    W1T = fW[0].T                 # [3, 128]
    SW1T = S @ W1T                # [9, 128]
    W1o3T = oW[0][:, 0:3].T       # [3, 128]
    SW1o3T = S @ W1o3T            # [9, 128]

    def fused_pair(phase, ymat, pmat, coefs, ncols):
        """Build (A, B) fused matrices: y block gets ymat [3,ncols] row-block,
        product block j gets coefs[j] * pmat (pmat [9,ncols] or [3,ncols])."""
        ka = KA1 if phase == 1 else KA2
        kb = KB1 if phase == 1 else KB2
        A = np.zeros((ka, ncols))
        Bm = np.zeros((kb, ncols))
        A[0:3] = ymat
        for j in range(1, 7):
            if coefs.get(j) is None:
                continue
            ti, r0, nr = _blk(phase, j)
            M = (A, Bm)[ti]
            M[r0:r0 + nr] = coefs[j] * pmat
        return A, Bm

    c = {}

    def put_pair(name, pair):
        c[name + "A"], c[name + "B"] = pair

    # stage-input fused matrices (-> layer-1 preact, ncols=128)
    for s in range(2, 7):
        coefs = {j: dt * _A[(s, j)] for j in range(1, s)}
        put_pair(f"st{s}_p1", fused_pair(1, W1T, SW1T, coefs, W))
        put_pair(f"st{s}_p2", fused_pair(2, W1o3T, W1o3T, coefs, W))
    bco = {j: dt * _B[j - 1] for j in range(1, 7)}
    # merged stage-1 (reads previous step's K-buffer)
    put_pair("st1m_p1", fused_pair(1, W1T, SW1T, bco, W))
    put_pair("st1m_p2", fused_pair(2, W1o3T, W1o3T, bco, W))
    put_pair("st1t_p2", fused_pair(1, W1o3T, SW1o3T, bco, W))  # transition
    # state update (-> y_{t+1}, ncols=3)
    put_pair("yu_p1", fused_pair(1, np.eye(3), S, bco, 3))
    put_pair("yu_p2", fused_pair(2, np.eye(3), np.eye(3), bco, 3))

    c["w1T_p1"] = W1T             # step-1 stage-1 (plain)
    c["w2T_p1"] = fW[1].T
    c["w3T_p1"] = fW[2].T
    c["w4T_p1"] = W4p.T           # [128, 9]
    c["b1_p1"] = fb[0][:, None]
    c["b2_p1"] = fb[1][:, None]
    c["b3_p1"] = fb[2][:, None]
    c["b4_p1"] = b4p[:, None]

    c["w2T_p2"] = oW[1].T
    c["w3T_p2"] = oW[2].T
    c["w4T_p2"] = oW[3].T         # [128, 3]
    c["b2_p2"] = ob[1][:, None]
    c["b3_p2"] = ob[2][:, None]
    c["b4_p2"] = ob[3][:, None]

    # phase-2 layer-1 bias table: col m*6+(s-1) = b1o + t_{m,s} * W1o[:,3]
    w1t_col = oW[0][:, 3]
    cols = []
    for m in range(N2):
        t0 = ts64[CU - 1 + m]
        dtm = ts64[CU + m] - ts64[CU - 1 + m]
        for s in range(1, 7):
            cols.append(ob[0] + (t0 + _C[s - 1] * dtm) * w1t_col)
    c["b1tab_p2"] = np.stack(cols, axis=1)   # [128, 6*N2]

    return {k: np.ascontiguousarray(v, np.float32) for k, v in c.items()}


def _control_precompute(ts, ys):
    """y0 [B,3] and permuted control derivative dxt [B, N1, 9]."""
    ys = np.asarray(ys, np.float32)
    cts = np.asarray(ts, np.float32)[:L]
    t_b = np.broadcast_to(cts, (ys.shape[0], L))
    Xa = np.stack([t_b, ys, t_b * ys], axis=-1).astype(np.float32)
    dX = (Xa[:, 1:] - Xa[:, :-1]) / (cts[1:] - cts[:-1])[None, :, None]
    y0 = Xa[:, 0]
    # dxt[b, g, r=3j+i] = dX[b, g, j]
    dxt = np.repeat(dX[:, :N1, :], 3, axis=2)
    return y0.astype(np.float32), dxt.astype(np.float32)


# --------------------------------------------------------------------------
# bass program
# --------------------------------------------------------------------------

_PROG_CACHE = {}


CHUNKS = 1


def build_program(n1=N1, n2=N2, prec=PRECISION, repeat=1, chunks=None):
    chunks = CHUNKS if chunks is None else chunks
    key = (n1, n2, prec, repeat, chunks)
    if key in _PROG_CACHE:
        return _PROG_CACHE[key]

    DT_W = F16 if prec == "mixed" else F32
    nc = bacc.Bacc("TRN2", target_bir_lowering=False, debug=False)

    const_specs = {
        "w1T_p1": [3, W],
        "w2T_p1": [W, W], "w3T_p1": [W, W], "w4T_p1": [W, 9],
        "b1_p1": [W, 1], "b2_p1": [W, 1], "b3_p1": [W, 1], "b4_p1": [9, 1],
        "w2T_p2": [W, W], "w3T_p2": [W, W], "w4T_p2": [W, 3],
        "b2_p2": [W, 1], "b3_p2": [W, 1], "b4_p2": [3, 1],
        "b1tab_p2": [W, 6 * N2],
        "y0": [3, BSH],
        "dxt": [9, N1 * BSH],
    }
    for s in range(2, 7):
        const_specs[f"st{s}_p1A"] = [KA1, W]
        const_specs[f"st{s}_p1B"] = [KB1, W]
        const_specs[f"st{s}_p2A"] = [KA2, W]
        const_specs[f"st{s}_p2B"] = [KB2, W]
    for nm in ("st1m_p1", "yu_p1", "st1t_p2"):
        cols = 3 if nm.startswith("yu") else W
        const_specs[nm + "A"] = [KA1, cols]
        const_specs[nm + "B"] = [KB1, cols]
    for nm in ("st1m_p2", "yu_p2"):
        cols = 3 if nm.startswith("yu") else W
        const_specs[nm + "A"] = [KA2, cols]
        const_specs[nm + "B"] = [KB2, cols]

    _wide = {"w2T_p1", "w3T_p1", "w4T_p1", "w2T_p2", "w3T_p2", "w4T_p2"}
    dram_in = {k: nc.dram_tensor(k, shp, DT_W if k in _wide else F32,
                                 kind="ExternalInput")
               for k, shp in const_specs.items()}
    out_d = nc.dram_tensor("out", [NSTEP, BSH], F32, kind="ExternalOutput")

    with tile.TileContext(nc) as tc:
        NCH = chunks
        CW = BSH // NCH
        with (
            tc.tile_pool(name="const", bufs=1) as constp,
            tc.tile_pool(name="act", bufs=2) as actp,
            tc.tile_pool(name="ps", bufs=(4 if NCH == 1 else 3),
                         space="PSUM") as psp,
            tc.tile_pool(name="psy", bufs=(2 if NCH == 1 else 1),
                         space="PSUM") as psyp,
        ):
            cst = {}
            for k, shp in const_specs.items():
                if k == "y0":
                    continue
                t = constp.tile(shp, DT_W if k in _wide else F32, tag=f"c_{k}")
                nc.sync.dma_start(t[:], dram_in[k][:])
                cst[k] = t

            # persistent K-buffer pairs: (phase, parity, chunk)
            kbuf = {}
            for ph, (ka, kb) in ((1, (KA1, KB1)), (2, (KA2, KB2))):
                for par in (0, 1):
                    for c in range(NCH):
                        a = constp.tile([ka, CW], F32, tag=f"kA{ph}_{par}_{c}")
                        b = constp.tile([kb, CW], F32, tag=f"kB{ph}_{par}_{c}")
                        nc.vector.memset(a[:], 0.0)
                        nc.vector.memset(b[:], 0.0)
                        kbuf[(ph, par, c)] = (a, b)
            # initial state into step-1's K-buffer
            for c in range(NCH):
                nc.sync.dma_start(kbuf[(1, 0, c)][0][0:3, :],
                                  dram_in["y0"][:, c * CW:(c + 1) * CW])

            def slot(g, c):
                ph = 1 if g < n1 else 2
                return ph, kbuf[(ph, g % 2, c)]

            def acc_pair(psum_ap, lhsA, lhsB, slotpair, crit):
                a, b = slotpair
                ops = [(lhsA, a), (lhsB, b)]
                first, second = (ops[1], ops[0]) if crit == 0 else (ops[0], ops[1])
                nc.tensor.matmul(psum_ap, first[0][:], first[1][:],
                                 start=True, stop=False)
                nc.tensor.matmul(psum_ap, second[0][:], second[1][:],
                                 start=False, stop=True)

            def ynew_block(g, c):
                """y_g for chunk c from slot g-1; store + save ch1."""
                php, pair_prev = slot(g - 1, c)
                if g < n1 + n2:
                    pair_cur = slot(g, c)[1]
                else:
                    pair_cur = None
                yu = "yu_p1" if php == 1 else "yu_p2"
                yn = psyp.tile([3, CW], F32, tag=f"ynew{c}")
                acc_pair(yn[:], cst[yu + "A"], cst[yu + "B"], pair_prev, crit=1)
                if pair_cur is not None:
                    dst = pair_cur[0]
                else:
                    dst = actp.tile([3, CW], F32, tag=f"yfin{c}")
                nc.vector.tensor_copy(dst[0:3, :], yn[:])
                nc.sync.dma_start(out_d[g - 1:g, c * CW:(c + 1) * CW],
                                  dst[1:2, :])

            # ---------------- steps ----------------
            CH = list(range(NCH))
            for _rep in range(repeat):
                for g in range(n1 + n2):
                    phase = 1 if g < n1 else 2
                    p = "p1" if phase == 1 else "p2"

                    # stage-1 pre-activations (merged form for g>=1)
                    pre1s = {}
                    for c in CH:
                        pre1s[c] = psp.tile([W, CW], F32, tag=f"pre{c}", name=f"pre1_{c}")
                        if g == 0:
                            pair = slot(g, c)[1]
                            nc.tensor.matmul(pre1s[c][:], cst["w1T_p1"][:],
                                             pair[0][0:3, :],
                                             start=True, stop=True)
                        else:
                            pair_prev = slot(g - 1, c)[1]
                            if g < n1:
                                nm = "st1m_p1"
                            elif g == n1:
                                nm = "st1t_p2"
                            else:
                                nm = "st1m_p2"
                            acc_pair(pre1s[c][:], cst[nm + "A"], cst[nm + "B"],
                                     pair_prev, crit=1)

                    # y_g materialization + save (off the critical path)
                    if g >= 1:
                        for c in CH:
                            ynew_block(g, c)

                    for s in range(1, 7):
                        if s >= 2:
                            ti_crit = _blk(phase, s - 1)[0]
                            for c in CH:
                                pre1s[c] = psp.tile([W, CW], F32, tag=f"pre{c}", name=f"pre1_{c}")
                                acc_pair(pre1s[c][:], cst[f"st{s}_{p}A"],
                                         cst[f"st{s}_{p}B"], slot(g, c)[1],
                                         crit=ti_crit)
                        if phase == 1:
                            bias1 = cst["b1_p1"][:, 0:1]
                        else:
                            col = 6 * (g - n1) + (s - 1)
                            bias1 = cst["b1tab_p2"][:, col:col + 1]

                        hs = {}
                        for c in CH:
                            hs[c] = actp.tile([W, CW], DT_W, tag=f"h1_{c}", name=f"h1_{c}")
                            nc.scalar.activation(hs[c][:], pre1s[c][:], Tanh,
                                                 bias=bias1, scale=1.0)
                        pres = {}
                        for c in CH:
                            pres[c] = psp.tile([W, CW], F32, tag=f"pre{c}", name=f"pre_{c}")
                            nc.tensor.matmul(pres[c][:], cst[f"w2T_{p}"][:],
                                             hs[c][:], start=True, stop=True)
                        for c in CH:
                            hs[c] = actp.tile([W, CW], DT_W, tag=f"h2_{c}", name=f"h2_{c}")
                            nc.scalar.activation(hs[c][:], pres[c][:], Tanh,
                                                 bias=cst[f"b2_{p}"][:, 0:1],
                                                 scale=1.0)
                        for c in CH:
                            pres[c] = psp.tile([W, CW], F32, tag=f"pre{c}", name=f"pre_{c}")
                            nc.tensor.matmul(pres[c][:], cst[f"w3T_{p}"][:],
                                             hs[c][:], start=True, stop=True)
                        for c in CH:
                            hs[c] = actp.tile([W, CW], DT_W, tag=f"h3_{c}", name=f"h3_{c}")
                            nc.scalar.activation(hs[c][:], pres[c][:], Tanh,
                                                 bias=cst[f"b3_{p}"][:, 0:1],
                                                 scale=1.0)
                        dout = 9 if phase == 1 else 3
                        for c in CH:
                            pres[c] = psp.tile([dout, CW], F32, tag=f"pre{c}", name=f"pre4_{c}")
                            nc.tensor.matmul(pres[c][:], cst[f"w4T_{p}"][:],
                                             hs[c][:], start=True, stop=True)
                        ti, r0, nr = _blk(phase, s)
                        for c in CH:
                            dst = slot(g, c)[1][ti]
                            if phase == 1:
                                mats = actp.tile([9, CW], F32, tag=f"mats{c}")
                                nc.scalar.activation(mats[:], pres[c][:], Tanh,
                                                     bias=cst["b4_p1"][:, 0:1],
                                                     scale=1.0)
                                nc.vector.tensor_mul(
                                    dst[r0:r0 + nr, :], mats[:],
                                    cst["dxt"][:, g * BSH + c * CW:
                                               g * BSH + (c + 1) * CW])
                            else:
                                nc.scalar.activation(dst[r0:r0 + nr, :],
                                                     pres[c][:], Tanh,
                                                     bias=cst["b4_p2"][:, 0:1],
                                                     scale=1.0)

                for c in CH:
                    ynew_block(n1 + n2, c)

    nc.compile()
    _PROG_CACHE[key] = nc
    return nc


# --------------------------------------------------------------------------
# host entry
# --------------------------------------------------------------------------

def _make_in_maps(ts, ys, func_params, ode_params, prec=PRECISION):
    consts = _fused_consts(func_params, ode_params, ts)
    if prec == "mixed":
        for k in ("w2T_p1", "w3T_p1", "w4T_p1", "w2T_p2", "w3T_p2", "w4T_p2"):
            consts[k] = consts[k].astype(np.float16)
    y0, dxt = _control_precompute(ts, ys)
    in_maps = []
    for cidx in range(NCORES):
        sl = slice(cidx * BSH, (cidx + 1) * BSH)
        m = dict(consts)
        m["y0"] = np.ascontiguousarray(y0[sl].T)
        m["dxt"] = np.ascontiguousarray(
            dxt[sl].transpose(2, 1, 0).reshape(9, N1 * BSH))
        in_maps.append(m)
    return in_maps


def run_on_hw(ts, ys, func_params, ode_params, n1=N1, n2=N2, trace=False,
              prec=PRECISION, chunks=None, **kw):
    nc = build_program(n1, n2, prec, 1, chunks)
    in_maps = _make_in_maps(ts, ys, func_params, ode_params, prec)
    res = run_bass_kernel_spmd(nc, in_maps, core_ids=list(range(NCORES)),
                               trace=trace, **kw)
    ys = np.asarray(ys, np.float32)
    out = np.empty((B, L), np.float32)
    out[:, 0] = ys[:, 0]
    for cidx in range(NCORES):
        sl = slice(cidx * BSH, (cidx + 1) * BSH)
        out[sl, 1:] = res.results[cidx]["out"].T
    return out, res


def kernel(ts, ys, control_until, saveat, train_until, func_params, ode_params):
    assert int(control_until) == CU and int(train_until) == TU
    out, _ = run_on_hw(ts, ys, func_params, ode_params, trace=False)
    return out
